# revision 27
# baseline (speedup 1.0000x reference)
"""Trainium2 Bass kernel for nn_MultiHeadAttention_52304111731071.

Sharding: 8 cores = 4 batches x 2 head-groups (tensor parallel over heads).
Each core computes q/k/v projections for its 512 channels (8 heads), partial
RoPE, full attention for its heads, and a partial O-projection; the host sums
the two partials per batch.

v7 (161.5us max-core neuron-profile; prior baseline 242.7us; rel err 2.7e-3).
Profile-driven design, measured on these cores:
  - dense back-to-back matmuls pace at ~216ns (N=512 bf16 warm, LDWEIGHTS and
    drain hidden); isolated matmuls cost 379ns - so the kernel is built as a
    single near-gapless PE stream with deep psum/sbuf buffering.
  - ACT exp costs (N+352)/1.2ns per instruction; 8.4M exps/core = 71.5us
    minimum.  Scores for two heads land in one [128, 2x512] psum tile
    spanning two banks and are exp'd by ONE ACT instruction (1114ns).
  - the exp stream starts at ~43us: K proj fully, then per-sub [Q-proj tile,
    scores+exp block] warmup interleaved with the V projection; PV for sub s
    lags scores by 2 subs so the PE never waits on ACT.
  - rope: sin table is pre-permuted on host (sin_perm[p] = sin[pi(p)]) so the
    rotate-half perm-matmul lands the final sin term: epilogue = evict
    (ACT Identity + per-partition bias AP early, DVE tensor_scalar_add during
    attention), sin-mult, perm matmul, cos-mult, add (3 DVE ops).
  - PV psum is evicted to SBUF immediately (frees po banks for the next sub;
    also the reciprocal_approx_fast custom op must read whole offset-0 SBUF
    tiles - offset APs or PSUM reads produce garbage on HW).
  - O-proj(n0) rides as filler inside attention n1 blocks; the O(n1) tail
    pre-accumulates kc0-2 across aux+sc psum slots while the final normalize
    chain runs, so only the kc3 matmuls wait on it; the last normalize skips
    the pvc eviction and reads po/psum directly (only the rcp custom op needs
    SBUF input).  Keeps the PE dense so the HAM clock-gate stays at 2.4GHz.
  - DMA order: wk/c(n0) interleaved, c(n1), then x0/wq BEFORE wv so the Q
    projection is never weight-gated.
PSUM budget (8 banks): scores 2x[128,1024] (4) + misc shp/po (2) + aux (2).

Measured dead ends: fp8e4m3 DoubleRow PV fails the 2e-2 gate (2.06e-2);
split-K PV (K=64 halves, per-member tile_position in one accumulation group)
compiles but dies at runtime with NRT INTERNAL; gpsimd copies in the
normalize chain are 4x slower than DVE and serialize with the broadcasts.
"""

import sys

sys.path.insert(0, "/opt/trn_rl_repo")

import numpy as np
import ml_dtypes

import concourse.bass as bass  # noqa: F401
import concourse.bacc as bacc
import concourse.mybir as mybir
import concourse.tile as tile

B, C, T, H = 4, 1024, 1024, 16
DH = 64
D_ROPE = 32
ROPE_BASE = 10000.0
P = 128
N_CORES = 8
HL = 8  # heads per core
CL = 512  # channels per core
KC = 8  # contraction subtiles (1024/128)
FP32 = mybir.dt.float32
BF16 = mybir.dt.bfloat16
SCALE = 1.0 / 8.0  # 1/sqrt(DH)
BF16NP = ml_dtypes.bfloat16


DEBUG_DUMP = False


def _build_program(repeat=1):
    nc = bacc.Bacc("TRN2", target_bir_lowering=False, debug=False)

    # all big inputs pre-laid-out on host so every DMA line is contiguous
    x_d = nc.dram_tensor("x_b", [P, 2, KC, 512], BF16, kind="ExternalInput")
    c_d = nc.dram_tensor("c_b", [P, 2, KC, 512], BF16, kind="ExternalInput")
    qwT_d = nc.dram_tensor("qwT", [P, KC, CL], BF16, kind="ExternalInput")
    kwT_d = nc.dram_tensor("kwT", [P, 4, KC, P], BF16, kind="ExternalInput")
    vwT_d = nc.dram_tensor("vwT", [P, KC, CL], BF16, kind="ExternalInput")
    owT_d = nc.dram_tensor("owT", [P, 4, C], BF16, kind="ExternalInput")
    qb_d = nc.dram_tensor("qb", [CL], FP32, kind="ExternalInput")
    kb_d = nc.dram_tensor("kb", [CL], FP32, kind="ExternalInput")
    ob_d = nc.dram_tensor("ob", [C], FP32, kind="ExternalInput")
    cos_d = nc.dram_tensor("cosr", [P, T], BF16, kind="ExternalInput")
    sin_d = nc.dram_tensor("sins", [P, T], BF16, kind="ExternalInput")
    pm_d = nc.dram_tensor("pm", [P, P], BF16, kind="ExternalInput")
    y_d = nc.dram_tensor("y", [C, T], FP32, kind="ExternalOutput")

    ID = mybir.ActivationFunctionType.Identity
    EXP = mybir.ActivationFunctionType.Exp
    MUL = mybir.AluOpType.mult
    ADD = mybir.AluOpType.add

    with tile.TileContext(nc) as tc:
      for _rep in range(repeat):
        with (
            tc.tile_pool(name="bigp", bufs=1) as bigp,
            tc.tile_pool(name="dynp", bufs=1) as dynp,
        ):
            # few pools on purpose: every pool adds a cross-engine barrier
            # set to the end-of-program teardown ladder (~60 semaphore waits
            # per engine at 15 pools, ~8us of pure epilogue)
            wq_p = wo_p = wk_p = wv_p = acts = consts = bigp
            stream = rope_p = exp_p = pvc_p = nrm_p = y_p = dynp
            # ---- big DMAs first: wk + c chunks interleaved so the first
            # K-proj matmuls can start after ~1/4 of the bytes land ----
            wk = wk_p.tile([P, 4, KC, P], BF16, tag="wk")
            ct0 = stream.tile([P, KC, 512], BF16, tag="stream", bufs=3)
            for kc in range(KC):
                nc.sync.dma_start(wk[:, 0, kc, :], kwT_d.ap()[:, 0, kc, :])
                nc.sync.dma_start(ct0[:, kc, :], c_d.ap()[:, 0, kc, :])
            for sub in range(1, 4):
                nc.sync.dma_start(wk[:, sub, :, :], kwT_d.ap()[:, sub, :, :])
            ct1 = stream.tile([P, KC, 512], BF16, tag="stream", bufs=3)
            nc.sync.dma_start(ct1[:], c_d.ap()[:, 1, :, :])
            cts = [ct0, ct1]
            x0 = stream.tile([P, KC, 512], BF16, tag="stream", bufs=3)
            nc.sync.dma_start(x0[:], x_d.ap()[:, 0, :, :])
            wq = wq_p.tile([P, KC, CL], BF16, tag="wq")
            nc.sync.dma_start(wq[:], qwT_d.ap())
            wv = wv_p.tile([P, KC, CL], BF16, tag="wv")
            nc.sync.dma_start(wv[:], vwT_d.ap())
            x1 = stream.tile([P, KC, 512], BF16, tag="stream", bufs=3)
            nc.sync.dma_start(x1[:], x_d.ap()[:, 1, :, :])
            xt = [x0, x1]
            wo = wo_p.tile([P, 4, T], BF16, tag="wo")
            nc.sync.dma_start(wo[:], owT_d.ap())

            # ---- tables / biases / permutation matrix ----
            cosr = consts.tile([P, T], BF16, tag="cosr")
            sins = consts.tile([P, T], BF16, tag="sins")
            nc.gpsimd.dma_start(cosr[:], cos_d.ap())
            nc.gpsimd.dma_start(sins[:], sin_d.ap())
            pm_sb = consts.tile([P, P], BF16, tag="pm")
            nc.gpsimd.dma_start(pm_sb[:], pm_d.ap())
            qb_sb = consts.tile([P, 4], FP32, tag="qb")
            kb_sb = consts.tile([P, 4], FP32, tag="kb")
            ob_sb = consts.tile([P, 8], FP32, tag="ob")
            nc.gpsimd.dma_start(qb_sb[:], qb_d.ap().rearrange("(s p) -> p s", p=P))
            nc.gpsimd.dma_start(kb_sb[:], kb_d.ap().rearrange("(s p) -> p s", p=P))
            nc.gpsimd.dma_start(ob_sb[:], ob_d.ap().rearrange("(s p) -> p s", p=P))

            q_sb = acts.tile([P, 4, T], BF16, tag="qsb")
            k_sb = acts.tile([P, 4, T], BF16, tag="ksb")
            vT_sb = acts.tile([P, KC, HL, 65], BF16, tag="vtsb")
            out_sb = acts.tile([P, 4, T], BF16, tag="osb")
            # ones column per head (col 64 of each 65-col group)
            ones_c = consts.tile([P, KC, 1], BF16, tag="ones")
            nc.any.memset(ones_c[:], 1.0)
            for j in range(HL):
                nc.vector.tensor_copy(vT_sb[:, :, j, 64:65], ones_c[:])

            with tc.tile_pool(name="psp", bufs=2, space="PSUM") as psp:
                sc_p = aux_p = misc_p = psp

                def rope_epilogue(dst, ps, bias_col, n, on_act):
                    """dst (128,512) bf16 slice of q/k subtile: bias + RoPE.

                    tmp = bf16(ps + bias); tmp_s = tmp*sin_perm (sin is
                    pre-permuted on host so the perm matmul lands the final
                    sin term); shp = pm @ tmp_s; dst = tmp*cos + shp.
                    """
                    ncol = slice(n * 512, (n + 1) * 512)
                    tmp = rope_p.tile([P, 512], BF16, tag="tmp", bufs=3)
                    if on_act:
                        nc.scalar.activation(tmp[:], ps[:], ID, bias=bias_col)
                    else:
                        nc.vector.tensor_scalar_add(tmp[:], ps[:], bias_col)
                    ts_ = rope_p.tile([P, 512], BF16, tag="ts", bufs=3)
                    nc.vector.tensor_tensor(ts_[:], tmp[:], sins[:, ncol], MUL)
                    shp = misc_p.tile([P, 512], FP32, tag="misc", name="shp")
                    nc.tensor.matmul(shp[:], pm_sb[:], ts_[:], start=True, stop=True)
                    nc.vector.tensor_tensor(dst, tmp[:], cosr[:, ncol], MUL)
                    nc.vector.tensor_tensor(dst, dst, shp[:], ADD)

                def k_tile(n, sub):
                    ps = aux_p.tile([P, 512], FP32, tag="aux", name="psk")
                    for kc in range(KC):
                        nc.tensor.matmul(
                            ps[:],
                            wk[:, sub, kc, :],
                            cts[n][:, kc, :],
                            start=(kc == 0),
                            stop=(kc == KC - 1),
                        )
                    rope_epilogue(
                        k_sb[:, sub, n * 512 : (n + 1) * 512],
                        ps,
                        kb_sb[:, sub : sub + 1],
                        n,
                        on_act=True,
                    )

                def q_tile(n, sub, on_act):
                    ps = aux_p.tile([P, 512], FP32, tag="aux", name="psq")
                    for kc in range(KC):
                        nc.tensor.matmul(
                            ps[:],
                            wq[:, kc, sub * P : (sub + 1) * P],
                            xt[n][:, kc, :],
                            start=(kc == 0),
                            stop=(kc == KC - 1),
                        )
                    rope_epilogue(
                        q_sb[:, sub, n * 512 : (n + 1) * 512],
                        ps,
                        qb_sb[:, sub : sub + 1],
                        n,
                        on_act=on_act,
                    )

                def v_quarter(mt):
                    ctile = cts[mt // 4]
                    toff = (mt % 4) * P
                    ps = aux_p.tile([P, HL, 64], FP32, tag="aux", name="psv")
                    for kc in range(KC):
                        nc.tensor.matmul(
                            ps[:],
                            ctile[:, kc, toff : toff + P],
                            wv[:, kc, :],
                            start=(kc == 0),
                            stop=(kc == KC - 1),
                        )
                    # one strided copy into the 65-stride vT layout
                    nc.vector.tensor_copy(vT_sb[:, mt, :, 0:64], ps[:])

                if DEBUG_DUMP:
                    _pvdbg = nc.dram_tensor(
                        "pvdbg", [2, 4, 2, 65, 512], FP32, kind="ExternalOutput"
                    )
                    _edbg = nc.dram_tensor(
                        "edbg", [KC, P, 2, 512], BF16, kind="ExternalOutput"
                    )

                # e tiles per (n, sub): list of 8 [P, 2, 512] bf16
                e_store = {}

                def scores_exp(n, sub, tk):
                    ncol = slice(n * 512, (n + 1) * 512)
                    sc = sc_p.tile([P, 2, 512], FP32, tag="sc", name="sc")
                    for half in range(2):
                        hb = half * 64
                        nc.tensor.matmul(
                            sc[:, half, :],
                            k_sb[hb : hb + 64, sub, tk * P : (tk + 1) * P],
                            q_sb[hb : hb + 64, sub, ncol],
                            start=True,
                            stop=True,
                            tile_position=(hb, 0),
                        )
                    e = exp_p.tile([P, 2, 512], BF16, name="e", tag="e", bufs=26)
                    nc.scalar.activation(e[:], sc[:], EXP, scale=SCALE)
                    e_store[(n, sub, tk)] = e
                    if DEBUG_DUMP and (n, sub) == (0, 0):
                        nc.sync.dma_start(_edbg.ap()[tk], e[:])

                po_store = {}

                def pv(n, sub, tk):
                    if tk == 0:
                        po_store[(n, sub)] = [
                            misc_p.tile([65, 512], FP32, tag="misc", name=f"po{h}")
                            for h in range(2)
                        ]
                    po = po_store[(n, sub)]
                    e = e_store[(n, sub, tk)]
                    # NOTE: split-K PV (K=64 halves on row groups 0/64 with
                    # per-group tile_position) crashes the NEFF at runtime
                    # (NRT INTERNAL) - keep full-K PV
                    for half in range(2):
                        j = 2 * sub + half
                        nc.tensor.matmul(
                            po[half][0:65, :],
                            vT_sb[:, tk, j, :],
                            e[:, half, :],
                            start=(tk == 0),
                            stop=(tk == KC - 1),
                        )

                def normalize(n, sub, last=False):
                    ncol = slice(n * 512, (n + 1) * 512)
                    po = po_store.pop((n, sub))
                    if last:
                        # final sub: skip the pvc eviction (nothing needs the
                        # po banks afterwards); den rows via regular DVE copies
                        # (safe from PSUM - only the rcp custom op is not)
                        den = nrm_p.tile([1, 1024], FP32, tag="den", bufs=2)
                        nc.vector.tensor_copy(den[:, 0:512], po[0][64:65, :])
                        nc.vector.tensor_copy(den[:, 512:1024], po[1][64:65, :])
                        rc = nrm_p.tile([1, 1024], FP32, tag="rc", bufs=2)
                        nc.vector.reciprocal_approx_fast(rc[:], den[:])
                        pbt = nrm_p.tile([64, 1024], FP32, tag="pbt", bufs=2)
                        nc.gpsimd.partition_broadcast(pbt[:], rc[:])
                        nc.vector.tensor_tensor(
                            out_sb[0:64, sub, ncol], po[0][0:64, :],
                            pbt[:, 0:512], MUL,
                        )
                        tmp1 = nrm_p.tile([64, 512], BF16, tag="t1", bufs=2)
                        nc.vector.tensor_tensor(
                            tmp1[:], po[1][0:64, :], pbt[:, 512:1024], MUL
                        )
                        nc.sync.dma_start(out_sb[64:128, sub, ncol], tmp1[:])
                        return
                    # evict PV psum to SBUF right away: frees the po banks for
                    # the next sub and keeps the rcp custom op off PSUM
                    pvc = []
                    for h in range(2):
                        t_ = pvc_p.tile([65, 512], FP32, name=f"pvc{h}", tag="pvc", bufs=4)
                        nc.vector.tensor_copy(t_[:], po[h][:])
                        pvc.append(t_)
                        if DEBUG_DUMP:
                            nc.sync.dma_start(_pvdbg.ap()[n, sub, h, :, :], t_[:])
                    # assemble both denominators into a fresh [1, 1024] tile:
                    # reciprocal_approx_fast is a custom DVE op - feed it only
                    # whole tiles at offset 0 (offset APs misbehave on HW)
                    den = nrm_p.tile([1, 1024], FP32, tag="den", bufs=2)
                    nc.vector.tensor_copy(den[:, 0:512], pvc[0][64:65, :])
                    nc.vector.tensor_copy(den[:, 512:1024], pvc[1][64:65, :])
                    rc = nrm_p.tile([1, 1024], FP32, tag="rc", bufs=2)
                    nc.vector.reciprocal_approx_fast(rc[:], den[:])
                    pbt = nrm_p.tile([64, 1024], FP32, tag="pbt", bufs=2)
                    nc.gpsimd.partition_broadcast(pbt[:], rc[:])
                    nc.vector.tensor_tensor(
                        out_sb[0:64, sub, ncol], pvc[0][0:64, :], pbt[:, 0:512], MUL
                    )
                    tmp1 = nrm_p.tile([64, 512], BF16, tag="t1", bufs=2)
                    nc.vector.tensor_tensor(tmp1[:], pvc[1][0:64, :], pbt[:, 512:1024], MUL)
                    # partition-shift via DMA on the (idle) sync queue
                    nc.sync.dma_start(out_sb[64:128, sub, ncol], tmp1[:])

                o_ps = {}

                def o_head(n, m, pool, tag, nkc):
                    # first nkc chunks of the contraction into a fresh psum tile
                    ncol = slice(n * 512, (n + 1) * 512)
                    ps = pool.tile([P, 512], FP32, tag=tag, name="pso")
                    o_ps[(n, m)] = ps
                    for kc in range(nkc):
                        nc.tensor.matmul(
                            ps[:],
                            wo[:, kc, m * P : (m + 1) * P],
                            out_sb[:, kc, ncol],
                            start=(kc == 0),
                            stop=False,
                        )

                def o_finish(n, m):
                    ncol = slice(n * 512, (n + 1) * 512)
                    ps = o_ps.pop((n, m))
                    nc.tensor.matmul(
                        ps[:],
                        wo[:, 3, m * P : (m + 1) * P],
                        out_sb[:, 3, ncol],
                        start=False,
                        stop=True,
                    )
                    ys = y_p.tile([P, 512], FP32, name="ys", tag="ys", bufs=3)
                    nc.vector.tensor_scalar_add(ys[:], ps[:], ob_sb[:, m : m + 1])
                    eng = nc.sync if m % 2 == 0 else nc.gpsimd
                    eng.dma_start(y_d.ap()[m * P : (m + 1) * P, ncol], ys[:])

                def o_tile(n, m):
                    o_head(n, m, aux_p, "aux", 3)
                    o_finish(n, m)

                def block(sc_ns, pv_ns):
                    """Lag-1 attention block: scores+exp for sc_ns while PV for
                    pv_ns accumulates; interleaved per tk."""
                    for tk in range(KC):
                        if sc_ns is not None:
                            scores_exp(sc_ns[0], sc_ns[1], tk)
                        if pv_ns is not None:
                            pv(pv_ns[0], pv_ns[1], tk)

                # ================= emission =================
                # projections: K fully, then Q(n0) tiles feeding the warm-up
                for n in range(2):
                    for sub in range(4):
                        k_tile(n, sub)
                q_tile(0, 0, on_act=True)
                block((0, 0), None)
                q_tile(0, 1, on_act=True)
                for mt in range(4):
                    v_quarter(mt)
                block((0, 1), None)
                # V q4-7 interleaved with PV(0,0): merges the two serial
                # phases that caused the exp-stream transition stall
                for mt in range(4, 8):
                    v_quarter(mt)
                    pv(0, 0, 2 * (mt - 4))
                    pv(0, 0, 2 * (mt - 4) + 1)
                normalize(0, 0)
                q_tile(0, 2, on_act=False)
                block((0, 2), None)
                q_tile(0, 3, on_act=False)

                # steady pipeline: PV lags scores by 2 subs
                blocks = [
                    ((0, 3), (0, 1), [lambda: q_tile(1, 0, on_act=False)]),
                    ((1, 0), (0, 2), [lambda: q_tile(1, 1, on_act=False)]),
                    ((1, 1), (0, 3), [lambda: q_tile(1, 2, on_act=False)]),
                    ((1, 2), (1, 0), [lambda: q_tile(1, 3, on_act=False),
                                      lambda: o_tile(0, 0)]),
                    ((1, 3), (1, 1), [lambda: o_tile(0, 1), lambda: o_tile(0, 2)]),
                    (None, (1, 2), [lambda: o_tile(0, 3), lambda: o_tile(0, 4),
                                    lambda: o_tile(0, 5), lambda: o_tile(0, 6),
                                    lambda: o_tile(0, 7)]),
                ]
                for sc_ns, pv_ns, fills in blocks:
                    block(sc_ns, pv_ns)
                    normalize(*pv_ns)
                    for f in fills:
                        f()

                # trailing PV(1,3) interleaved with the remaining O(n0) tiles
                # and O(n1) kc0-2 pre-accumulation, so the PE stays dense (and
                # the HAM clock warm) through the tail; only the kc3 matmuls
                # wait on the final normalize
                for tk in range(KC):
                    pv(1, 3, tk)
                # kc0-2 pre-accumulation overlaps the final normalize chain
                o_head(1, 0, aux_p, "aux", 3)
                o_head(1, 1, aux_p, "aux", 3)
                o_head(1, 2, sc_p, "sc", 3)
                o_head(1, 3, sc_p, "sc", 3)
                normalize(1, 3, last=True)
                o_finish(1, 0)
                o_finish(1, 1)
                o_tile(1, 4)
                o_finish(1, 2)
                o_tile(1, 5)
                o_finish(1, 3)
                o_tile(1, 6)
                o_tile(1, 7)

    nc.compile()
    return nc


def _rope_tables():
    theta = 1.0 / (ROPE_BASE ** (np.arange(0, D_ROPE, 2, dtype=np.float32) / D_ROPE))
    ang = np.arange(T, dtype=np.float32)[:, None] * theta[None, :]  # (T, 16)
    ang2 = np.concatenate([ang, ang], axis=1)  # (T, 32)
    cos2 = np.cos(ang2).astype(np.float32)  # (T, 32)
    sin2 = np.sin(ang2).astype(np.float32)
    cosr = np.ones((P, T), np.float32)
    sins = np.zeros((P, T), np.float32)
    for base in (0, 64):
        for d in range(D_ROPE):
            cosr[base + d] = cos2[:, d]
            # permuted sin: row p holds sin[pi(p)] where pi swaps d <-> d+16
            # within each 32-row rope block, so that
            # (pm @ (tmp * sin_perm))[r] = sign_r * tmp[pi(r)] * sin[r]
            dp = d + 16 if d < 16 else d - 16
            sins[base + d] = sin2[:, dp]
    # permutation matrix: sh = pm.T @ tmp; sh[g+i] = -tmp[g+16+i],
    # sh[g+16+i] = +tmp[g+i] for i in 0:16, g in {0,64}; zero elsewhere
    pm = np.zeros((P, P), np.float32)
    for g in (0, 64):
        for i in range(16):
            pm[g + 16 + i, g + i] = -1.0
            pm[g + i, g + 16 + i] = 1.0
    return cosr, sins, pm


def make_in_maps(x, c, q_w, q_b, kv_w, kv_b, o_w, o_b):
    x = np.asarray(x, np.float32)
    c = np.asarray(c, np.float32)
    q_w = np.asarray(q_w, np.float32)
    q_b = np.asarray(q_b, np.float32)
    kv_w = np.asarray(kv_w, np.float32)
    kv_b = np.asarray(kv_b, np.float32)
    o_w = np.asarray(o_w, np.float32)
    o_b = np.asarray(o_b, np.float32)
    cosr, sins, pm = _rope_tables()

    def act_layout(a):  # (C, T) -> (P, 2, KC, 512): [p][n][ko][t]
        return np.ascontiguousarray(
            a.reshape(KC, P, 2, 512).transpose(1, 2, 0, 3)
        ).astype(BF16NP)

    def w_layout(wT):  # (C, CL) -> (P, KC, CL): [p][ko][m]
        ko = wT.shape[0] // P
        return np.ascontiguousarray(
            wT.reshape(ko, P, wT.shape[1]).transpose(1, 0, 2)
        ).astype(BF16NP)

    in_maps = []
    for core in range(N_CORES):
        b, g = core // 2, core % 2
        ch = slice(g * CL, (g + 1) * CL)
        ob_eff = o_w[:, ch] @ kv_b[C + g * CL : C + (g + 1) * CL]
        if g == 0:
            ob_eff = ob_eff + o_b
        kwT = kv_w[ch, :].T  # (C, CL): [ko*128+p, sub*128+j] -> [p][sub][ko][j]
        kwT4 = np.ascontiguousarray(
            kwT.reshape(KC, P, 4, P).transpose(1, 2, 0, 3)
        ).astype(BF16NP)
        in_maps.append(
            {
                "x_b": act_layout(x[b]),
                "c_b": act_layout(c[b]),
                "qwT": w_layout(q_w[ch, :].T),
                "kwT": kwT4,
                "vwT": w_layout(kv_w[C + g * CL : C + (g + 1) * CL, :].T),
                "owT": w_layout(o_w[:, ch].T),
                "qb": np.ascontiguousarray(q_b[ch]),
                "kb": np.ascontiguousarray(kv_b[ch]),
                "ob": np.ascontiguousarray(ob_eff.astype(np.float32)),
                "cosr": cosr.astype(BF16NP),
                "sins": sins.astype(BF16NP),
                "pm": pm.astype(BF16NP),
            }
        )
    return in_maps


_NC = None


def _get_nc():
    global _NC
    if _NC is None:
        _NC = _build_program()
    return _NC


def kernel(x, c, q_w, q_b, kv_w, kv_b, o_w, o_b):
    from concourse.bass_utils import run_bass_kernel_spmd

    nc = _get_nc()
    in_maps = make_in_maps(x, c, q_w, q_b, kv_w, kv_b, o_w, o_b)
    res = run_bass_kernel_spmd(nc, in_maps, core_ids=list(range(N_CORES)))
    y = np.empty((B, C, T), np.float32)
    for b in range(B):
        y[b] = res.results[2 * b]["y"] + res.results[2 * b + 1]["y"]
    return y


# revision 28
# speedup vs baseline: 1.1602x; 1.1602x over previous
"""Trainium2 Bass kernel for nn_MultiHeadAttention_52304111731071.

Sharding: 8 cores = 4 batches x 2 head-groups (tensor parallel over heads).
Each core computes q/k/v projections for its 512 channels (8 heads), partial
RoPE, full attention for its heads, and a partial O-projection; the host sums
the two partials per batch.

v7 (161.5us max-core neuron-profile; prior baseline 242.7us; rel err 2.7e-3).
Profile-driven design, measured on these cores:
  - dense back-to-back matmuls pace at ~216ns (N=512 bf16 warm, LDWEIGHTS and
    drain hidden); isolated matmuls cost 379ns - so the kernel is built as a
    single near-gapless PE stream with deep psum/sbuf buffering.
  - ACT exp costs (N+352)/1.2ns per instruction; 8.4M exps/core = 71.5us
    minimum.  Scores for two heads land in one [128, 2x512] psum tile
    spanning two banks and are exp'd by ONE ACT instruction (1114ns).
  - the exp stream starts at ~43us: K proj fully, then per-sub [Q-proj tile,
    scores+exp block] warmup interleaved with the V projection; PV for sub s
    lags scores by 2 subs so the PE never waits on ACT.
  - rope: sin table is pre-permuted on host (sin_perm[p] = sin[pi(p)]) so the
    rotate-half perm-matmul lands the final sin term: epilogue = evict
    (ACT Identity + per-partition bias AP early, DVE tensor_scalar_add during
    attention), sin-mult, perm matmul, cos-mult, add (3 DVE ops).
  - PV psum is evicted to SBUF immediately (frees po banks for the next sub;
    also the reciprocal_approx_fast custom op must read whole offset-0 SBUF
    tiles - offset APs or PSUM reads produce garbage on HW).
  - O-proj(n0) rides as filler inside attention n1 blocks; the O(n1) tail
    pre-accumulates kc0-2 across aux+sc psum slots while the final normalize
    chain runs, so only the kc3 matmuls wait on it; the last normalize skips
    the pvc eviction and reads po/psum directly (only the rcp custom op needs
    SBUF input).  Keeps the PE dense so the HAM clock-gate stays at 2.4GHz.
  - DMA order: wk/c(n0) interleaved, c(n1), then x0/wq BEFORE wv so the Q
    projection is never weight-gated.
PSUM budget (8 banks): scores 2x[128,1024] (4) + misc shp/po (2) + aux (2).

Measured dead ends: fp8e4m3 DoubleRow PV fails the 2e-2 gate (2.06e-2);
split-K PV (K=64 halves, per-member tile_position in one accumulation group)
compiles but dies at runtime with NRT INTERNAL; gpsimd copies in the
normalize chain are 4x slower than DVE and serialize with the broadcasts.
"""

import sys

sys.path.insert(0, "/opt/trn_rl_repo")

import numpy as np
import ml_dtypes

import concourse.bass as bass  # noqa: F401
import concourse.bacc as bacc
import concourse.mybir as mybir
import concourse.tile as tile

B, C, T, H = 4, 1024, 1024, 16
DH = 64
D_ROPE = 32
ROPE_BASE = 10000.0
P = 128
N_CORES = 8
HL = 8  # heads per core
CL = 512  # channels per core
KC = 8  # contraction subtiles (1024/128)
FP32 = mybir.dt.float32
BF16 = mybir.dt.bfloat16
SCALE = 1.0 / 8.0  # 1/sqrt(DH)
BF16NP = ml_dtypes.bfloat16


DEBUG_DUMP = False


def _build_program(repeat=1):
    nc = bacc.Bacc("TRN2", target_bir_lowering=False, debug=False)

    # all big inputs pre-laid-out on host so every DMA line is contiguous
    x_d = nc.dram_tensor("x_b", [P, 2, KC, 512], BF16, kind="ExternalInput")
    c_d = nc.dram_tensor("c_b", [P, 2, KC, 512], BF16, kind="ExternalInput")
    qwT_d = nc.dram_tensor("qwT", [P, KC, CL], BF16, kind="ExternalInput")
    kwT_d = nc.dram_tensor("kwT", [P, 4, KC, P], BF16, kind="ExternalInput")
    vwT_d = nc.dram_tensor("vwT", [P, KC, CL], BF16, kind="ExternalInput")
    owT_d = nc.dram_tensor("owT", [P, 4, C], BF16, kind="ExternalInput")
    qb_d = nc.dram_tensor("qb", [CL], FP32, kind="ExternalInput")
    kb_d = nc.dram_tensor("kb", [CL], FP32, kind="ExternalInput")
    ob_d = nc.dram_tensor("ob", [C], FP32, kind="ExternalInput")
    cos_d = nc.dram_tensor("cosr", [P, T], BF16, kind="ExternalInput")
    sin_d = nc.dram_tensor("sins", [P, T], BF16, kind="ExternalInput")
    pm_d = nc.dram_tensor("pm", [P, P], BF16, kind="ExternalInput")
    y_d = nc.dram_tensor("y", [C, T], FP32, kind="ExternalOutput")

    ID = mybir.ActivationFunctionType.Identity
    EXP = mybir.ActivationFunctionType.Exp
    MUL = mybir.AluOpType.mult
    ADD = mybir.AluOpType.add

    with tile.TileContext(nc) as tc:
      for _rep in range(repeat):
        with (
            tc.tile_pool(name="wq", bufs=1) as wq_p,
            tc.tile_pool(name="wo", bufs=1) as wo_p,
            tc.tile_pool(name="wk", bufs=1) as wk_p,
            tc.tile_pool(name="wv", bufs=1) as wv_p,
            tc.tile_pool(name="acts", bufs=1) as acts,
            tc.tile_pool(name="consts", bufs=1) as consts,
            tc.tile_pool(name="stream", bufs=3) as stream,
            tc.tile_pool(name="rope", bufs=3) as rope_p,
            tc.tile_pool(name="exp", bufs=26) as exp_p,
            tc.tile_pool(name="pvc", bufs=4) as pvc_p,
            tc.tile_pool(name="nrm", bufs=2) as nrm_p,
            tc.tile_pool(name="ysb", bufs=3) as y_p,
        ):
            # ---- big DMAs first: wk + c chunks interleaved so the first
            # K-proj matmuls can start after ~1/4 of the bytes land ----
            wk = wk_p.tile([P, 4, KC, P], BF16, tag="wk")
            ct0 = stream.tile([P, KC, 512], BF16, tag="stream", bufs=3)
            for kc in range(KC):
                nc.sync.dma_start(wk[:, 0, kc, :], kwT_d.ap()[:, 0, kc, :])
                nc.sync.dma_start(ct0[:, kc, :], c_d.ap()[:, 0, kc, :])
            for sub in range(1, 4):
                nc.sync.dma_start(wk[:, sub, :, :], kwT_d.ap()[:, sub, :, :])
            ct1 = stream.tile([P, KC, 512], BF16, tag="stream", bufs=3)
            nc.sync.dma_start(ct1[:], c_d.ap()[:, 1, :, :])
            cts = [ct0, ct1]
            x0 = stream.tile([P, KC, 512], BF16, tag="stream", bufs=3)
            nc.sync.dma_start(x0[:], x_d.ap()[:, 0, :, :])
            wq = wq_p.tile([P, KC, CL], BF16, tag="wq")
            nc.sync.dma_start(wq[:], qwT_d.ap())
            wv = wv_p.tile([P, KC, CL], BF16, tag="wv")
            nc.sync.dma_start(wv[:], vwT_d.ap())
            x1 = stream.tile([P, KC, 512], BF16, tag="stream", bufs=3)
            nc.sync.dma_start(x1[:], x_d.ap()[:, 1, :, :])
            xt = [x0, x1]
            wo = wo_p.tile([P, 4, T], BF16, tag="wo")
            nc.sync.dma_start(wo[:], owT_d.ap())

            # ---- tables / biases / permutation matrix ----
            cosr = consts.tile([P, T], BF16, tag="cosr")
            sins = consts.tile([P, T], BF16, tag="sins")
            nc.gpsimd.dma_start(cosr[:], cos_d.ap())
            nc.gpsimd.dma_start(sins[:], sin_d.ap())
            pm_sb = consts.tile([P, P], BF16, tag="pm")
            nc.gpsimd.dma_start(pm_sb[:], pm_d.ap())
            qb_sb = consts.tile([P, 4], FP32, tag="qb")
            kb_sb = consts.tile([P, 4], FP32, tag="kb")
            ob_sb = consts.tile([P, 8], FP32, tag="ob")
            nc.gpsimd.dma_start(qb_sb[:], qb_d.ap().rearrange("(s p) -> p s", p=P))
            nc.gpsimd.dma_start(kb_sb[:], kb_d.ap().rearrange("(s p) -> p s", p=P))
            nc.gpsimd.dma_start(ob_sb[:], ob_d.ap().rearrange("(s p) -> p s", p=P))

            q_sb = acts.tile([P, 4, T], BF16, tag="qsb")
            k_sb = acts.tile([P, 4, T], BF16, tag="ksb")
            vT_sb = acts.tile([P, KC, HL, 65], BF16, tag="vtsb")
            out_sb = acts.tile([P, 4, T], BF16, tag="osb")
            # ones column per head (col 64 of each 65-col group)
            ones_c = consts.tile([P, KC, 1], BF16, tag="ones")
            nc.any.memset(ones_c[:], 1.0)
            for j in range(HL):
                nc.vector.tensor_copy(vT_sb[:, :, j, 64:65], ones_c[:])

            with (
                tc.tile_pool(name="scp", bufs=2, space="PSUM") as sc_p,
                tc.tile_pool(name="auxp", bufs=2, space="PSUM") as aux_p,
                tc.tile_pool(name="miscp", bufs=2, space="PSUM") as misc_p,
            ):

                def rope_epilogue(dst, ps, bias_col, n, on_act):
                    """dst (128,512) bf16 slice of q/k subtile: bias + RoPE.

                    tmp = bf16(ps + bias); tmp_s = tmp*sin_perm (sin is
                    pre-permuted on host so the perm matmul lands the final
                    sin term); shp = pm @ tmp_s; dst = tmp*cos + shp.
                    """
                    ncol = slice(n * 512, (n + 1) * 512)
                    tmp = rope_p.tile([P, 512], BF16, tag="tmp", bufs=3)
                    if on_act:
                        nc.scalar.activation(tmp[:], ps[:], ID, bias=bias_col)
                    else:
                        nc.vector.tensor_scalar_add(tmp[:], ps[:], bias_col)
                    ts_ = rope_p.tile([P, 512], BF16, tag="ts", bufs=3)
                    nc.vector.tensor_tensor(ts_[:], tmp[:], sins[:, ncol], MUL)
                    shp = misc_p.tile([P, 512], FP32, tag="misc", name="shp")
                    nc.tensor.matmul(shp[:], pm_sb[:], ts_[:], start=True, stop=True)
                    nc.vector.tensor_tensor(dst, tmp[:], cosr[:, ncol], MUL)
                    nc.vector.tensor_tensor(dst, dst, shp[:], ADD)

                def k_tile(n, sub):
                    ps = aux_p.tile([P, 512], FP32, tag="aux", name="psk")
                    for kc in range(KC):
                        nc.tensor.matmul(
                            ps[:],
                            wk[:, sub, kc, :],
                            cts[n][:, kc, :],
                            start=(kc == 0),
                            stop=(kc == KC - 1),
                        )
                    rope_epilogue(
                        k_sb[:, sub, n * 512 : (n + 1) * 512],
                        ps,
                        kb_sb[:, sub : sub + 1],
                        n,
                        on_act=True,
                    )

                def q_tile(n, sub, on_act):
                    ps = aux_p.tile([P, 512], FP32, tag="aux", name="psq")
                    for kc in range(KC):
                        nc.tensor.matmul(
                            ps[:],
                            wq[:, kc, sub * P : (sub + 1) * P],
                            xt[n][:, kc, :],
                            start=(kc == 0),
                            stop=(kc == KC - 1),
                        )
                    rope_epilogue(
                        q_sb[:, sub, n * 512 : (n + 1) * 512],
                        ps,
                        qb_sb[:, sub : sub + 1],
                        n,
                        on_act=on_act,
                    )

                def v_quarter(mt):
                    ctile = cts[mt // 4]
                    toff = (mt % 4) * P
                    ps = aux_p.tile([P, HL, 64], FP32, tag="aux", name="psv")
                    for kc in range(KC):
                        nc.tensor.matmul(
                            ps[:],
                            ctile[:, kc, toff : toff + P],
                            wv[:, kc, :],
                            start=(kc == 0),
                            stop=(kc == KC - 1),
                        )
                    # one strided copy into the 65-stride vT layout
                    nc.vector.tensor_copy(vT_sb[:, mt, :, 0:64], ps[:])

                if DEBUG_DUMP:
                    _pvdbg = nc.dram_tensor(
                        "pvdbg", [2, 4, 2, 65, 512], FP32, kind="ExternalOutput"
                    )
                    _edbg = nc.dram_tensor(
                        "edbg", [KC, P, 2, 512], BF16, kind="ExternalOutput"
                    )

                # e tiles per (n, sub): list of 8 [P, 2, 512] bf16
                e_store = {}

                def scores_exp(n, sub, tk):
                    ncol = slice(n * 512, (n + 1) * 512)
                    sc = sc_p.tile([P, 2, 512], FP32, tag="sc", name="sc")
                    for half in range(2):
                        hb = half * 64
                        nc.tensor.matmul(
                            sc[:, half, :],
                            k_sb[hb : hb + 64, sub, tk * P : (tk + 1) * P],
                            q_sb[hb : hb + 64, sub, ncol],
                            start=True,
                            stop=True,
                            tile_position=(hb, 0),
                        )
                    e = exp_p.tile([P, 2, 512], BF16, name="e", tag="e", bufs=26)
                    nc.scalar.activation(e[:], sc[:], EXP, scale=SCALE)
                    e_store[(n, sub, tk)] = e
                    if DEBUG_DUMP and (n, sub) == (0, 0):
                        nc.sync.dma_start(_edbg.ap()[tk], e[:])

                po_store = {}

                def pv(n, sub, tk):
                    if tk == 0:
                        po_store[(n, sub)] = [
                            misc_p.tile([65, 512], FP32, tag="misc", name=f"po{h}")
                            for h in range(2)
                        ]
                    po = po_store[(n, sub)]
                    e = e_store[(n, sub, tk)]
                    # NOTE: split-K PV (K=64 halves on row groups 0/64 with
                    # per-group tile_position) crashes the NEFF at runtime
                    # (NRT INTERNAL) - keep full-K PV
                    for half in range(2):
                        j = 2 * sub + half
                        nc.tensor.matmul(
                            po[half][0:65, :],
                            vT_sb[:, tk, j, :],
                            e[:, half, :],
                            start=(tk == 0),
                            stop=(tk == KC - 1),
                        )

                def normalize(n, sub, last=False):
                    ncol = slice(n * 512, (n + 1) * 512)
                    po = po_store.pop((n, sub))
                    if last:
                        # final sub: skip the pvc eviction (nothing needs the
                        # po banks afterwards); den rows via regular DVE copies
                        # (safe from PSUM - only the rcp custom op is not)
                        den = nrm_p.tile([1, 1024], FP32, tag="den", bufs=2)
                        nc.vector.tensor_copy(den[:, 0:512], po[0][64:65, :])
                        nc.vector.tensor_copy(den[:, 512:1024], po[1][64:65, :])
                        rc = nrm_p.tile([1, 1024], FP32, tag="rc", bufs=2)
                        nc.vector.reciprocal_approx_fast(rc[:], den[:])
                        pbt = nrm_p.tile([64, 1024], FP32, tag="pbt", bufs=2)
                        nc.gpsimd.partition_broadcast(pbt[:], rc[:])
                        nc.vector.tensor_tensor(
                            out_sb[0:64, sub, ncol], po[0][0:64, :],
                            pbt[:, 0:512], MUL,
                        )
                        tmp1 = nrm_p.tile([64, 512], BF16, tag="t1", bufs=2)
                        nc.vector.tensor_tensor(
                            tmp1[:], po[1][0:64, :], pbt[:, 512:1024], MUL
                        )
                        nc.sync.dma_start(out_sb[64:128, sub, ncol], tmp1[:])
                        return
                    # evict PV psum to SBUF right away: frees the po banks for
                    # the next sub and keeps the rcp custom op off PSUM
                    pvc = []
                    for h in range(2):
                        t_ = pvc_p.tile([65, 512], FP32, name=f"pvc{h}", tag="pvc", bufs=4)
                        nc.vector.tensor_copy(t_[:], po[h][:])
                        pvc.append(t_)
                        if DEBUG_DUMP:
                            nc.sync.dma_start(_pvdbg.ap()[n, sub, h, :, :], t_[:])
                    # assemble both denominators into a fresh [1, 1024] tile:
                    # reciprocal_approx_fast is a custom DVE op - feed it only
                    # whole tiles at offset 0 (offset APs misbehave on HW)
                    den = nrm_p.tile([1, 1024], FP32, tag="den", bufs=2)
                    nc.vector.tensor_copy(den[:, 0:512], pvc[0][64:65, :])
                    nc.vector.tensor_copy(den[:, 512:1024], pvc[1][64:65, :])
                    rc = nrm_p.tile([1, 1024], FP32, tag="rc", bufs=2)
                    nc.vector.reciprocal_approx_fast(rc[:], den[:])
                    pbt = nrm_p.tile([64, 1024], FP32, tag="pbt", bufs=2)
                    nc.gpsimd.partition_broadcast(pbt[:], rc[:])
                    nc.vector.tensor_tensor(
                        out_sb[0:64, sub, ncol], pvc[0][0:64, :], pbt[:, 0:512], MUL
                    )
                    tmp1 = nrm_p.tile([64, 512], BF16, tag="t1", bufs=2)
                    nc.vector.tensor_tensor(tmp1[:], pvc[1][0:64, :], pbt[:, 512:1024], MUL)
                    # partition-shift via DMA on the (idle) sync queue
                    nc.sync.dma_start(out_sb[64:128, sub, ncol], tmp1[:])

                o_ps = {}

                def o_head(n, m, pool, tag, nkc):
                    # first nkc chunks of the contraction into a fresh psum tile
                    ncol = slice(n * 512, (n + 1) * 512)
                    ps = pool.tile([P, 512], FP32, tag=tag, name="pso")
                    o_ps[(n, m)] = ps
                    for kc in range(nkc):
                        nc.tensor.matmul(
                            ps[:],
                            wo[:, kc, m * P : (m + 1) * P],
                            out_sb[:, kc, ncol],
                            start=(kc == 0),
                            stop=False,
                        )

                def o_finish(n, m):
                    ncol = slice(n * 512, (n + 1) * 512)
                    ps = o_ps.pop((n, m))
                    nc.tensor.matmul(
                        ps[:],
                        wo[:, 3, m * P : (m + 1) * P],
                        out_sb[:, 3, ncol],
                        start=False,
                        stop=True,
                    )
                    ys = y_p.tile([P, 512], FP32, name="ys", tag="ys", bufs=3)
                    nc.vector.tensor_scalar_add(ys[:], ps[:], ob_sb[:, m : m + 1])
                    nc.sync.dma_start(y_d.ap()[m * P : (m + 1) * P, ncol], ys[:])

                def o_tile(n, m):
                    o_head(n, m, aux_p, "aux", 3)
                    o_finish(n, m)

                def block(sc_ns, pv_ns):
                    """Lag-1 attention block: scores+exp for sc_ns while PV for
                    pv_ns accumulates; interleaved per tk."""
                    for tk in range(KC):
                        if sc_ns is not None:
                            scores_exp(sc_ns[0], sc_ns[1], tk)
                        if pv_ns is not None:
                            pv(pv_ns[0], pv_ns[1], tk)

                # ================= emission =================
                # projections: K fully, then Q(n0) tiles feeding the warm-up
                for n in range(2):
                    for sub in range(4):
                        k_tile(n, sub)
                q_tile(0, 0, on_act=True)
                block((0, 0), None)
                q_tile(0, 1, on_act=True)
                for mt in range(4):
                    v_quarter(mt)
                block((0, 1), None)
                # V q4-7 interleaved with PV(0,0): merges the two serial
                # phases that caused the exp-stream transition stall
                for mt in range(4, 8):
                    v_quarter(mt)
                    pv(0, 0, 2 * (mt - 4))
                    pv(0, 0, 2 * (mt - 4) + 1)
                normalize(0, 0)
                q_tile(0, 2, on_act=False)
                block((0, 2), None)
                q_tile(0, 3, on_act=False)

                # steady pipeline: PV lags scores by 2 subs
                blocks = [
                    ((0, 3), (0, 1), [lambda: q_tile(1, 0, on_act=False)]),
                    ((1, 0), (0, 2), [lambda: q_tile(1, 1, on_act=False)]),
                    ((1, 1), (0, 3), [lambda: q_tile(1, 2, on_act=False)]),
                    ((1, 2), (1, 0), [lambda: q_tile(1, 3, on_act=False),
                                      lambda: o_tile(0, 0)]),
                    ((1, 3), (1, 1), [lambda: o_tile(0, 1), lambda: o_tile(0, 2)]),
                    (None, (1, 2), [lambda: o_tile(0, 3), lambda: o_tile(0, 4),
                                    lambda: o_tile(0, 5), lambda: o_tile(0, 6),
                                    lambda: o_tile(0, 7)]),
                ]
                for sc_ns, pv_ns, fills in blocks:
                    block(sc_ns, pv_ns)
                    normalize(*pv_ns)
                    for f in fills:
                        f()

                # trailing PV(1,3) interleaved with the remaining O(n0) tiles
                # and O(n1) kc0-2 pre-accumulation, so the PE stays dense (and
                # the HAM clock warm) through the tail; only the kc3 matmuls
                # wait on the final normalize
                for tk in range(KC):
                    pv(1, 3, tk)
                # kc0-2 pre-accumulation overlaps the final normalize chain
                o_head(1, 0, aux_p, "aux", 3)
                o_head(1, 1, aux_p, "aux", 3)
                o_head(1, 2, sc_p, "sc", 3)
                o_head(1, 3, sc_p, "sc", 3)
                normalize(1, 3, last=True)
                o_finish(1, 0)
                o_finish(1, 1)
                o_tile(1, 4)
                o_finish(1, 2)
                o_tile(1, 5)
                o_finish(1, 3)
                o_tile(1, 6)
                o_tile(1, 7)

    nc.compile()
    return nc


def _rope_tables():
    theta = 1.0 / (ROPE_BASE ** (np.arange(0, D_ROPE, 2, dtype=np.float32) / D_ROPE))
    ang = np.arange(T, dtype=np.float32)[:, None] * theta[None, :]  # (T, 16)
    ang2 = np.concatenate([ang, ang], axis=1)  # (T, 32)
    cos2 = np.cos(ang2).astype(np.float32)  # (T, 32)
    sin2 = np.sin(ang2).astype(np.float32)
    cosr = np.ones((P, T), np.float32)
    sins = np.zeros((P, T), np.float32)
    for base in (0, 64):
        for d in range(D_ROPE):
            cosr[base + d] = cos2[:, d]
            # permuted sin: row p holds sin[pi(p)] where pi swaps d <-> d+16
            # within each 32-row rope block, so that
            # (pm @ (tmp * sin_perm))[r] = sign_r * tmp[pi(r)] * sin[r]
            dp = d + 16 if d < 16 else d - 16
            sins[base + d] = sin2[:, dp]
    # permutation matrix: sh = pm.T @ tmp; sh[g+i] = -tmp[g+16+i],
    # sh[g+16+i] = +tmp[g+i] for i in 0:16, g in {0,64}; zero elsewhere
    pm = np.zeros((P, P), np.float32)
    for g in (0, 64):
        for i in range(16):
            pm[g + 16 + i, g + i] = -1.0
            pm[g + i, g + 16 + i] = 1.0
    return cosr, sins, pm


def make_in_maps(x, c, q_w, q_b, kv_w, kv_b, o_w, o_b):
    x = np.asarray(x, np.float32)
    c = np.asarray(c, np.float32)
    q_w = np.asarray(q_w, np.float32)
    q_b = np.asarray(q_b, np.float32)
    kv_w = np.asarray(kv_w, np.float32)
    kv_b = np.asarray(kv_b, np.float32)
    o_w = np.asarray(o_w, np.float32)
    o_b = np.asarray(o_b, np.float32)
    cosr, sins, pm = _rope_tables()

    def act_layout(a):  # (C, T) -> (P, 2, KC, 512): [p][n][ko][t]
        return np.ascontiguousarray(
            a.reshape(KC, P, 2, 512).transpose(1, 2, 0, 3)
        ).astype(BF16NP)

    def w_layout(wT):  # (C, CL) -> (P, KC, CL): [p][ko][m]
        ko = wT.shape[0] // P
        return np.ascontiguousarray(
            wT.reshape(ko, P, wT.shape[1]).transpose(1, 0, 2)
        ).astype(BF16NP)

    in_maps = []
    for core in range(N_CORES):
        b, g = core // 2, core % 2
        ch = slice(g * CL, (g + 1) * CL)
        ob_eff = o_w[:, ch] @ kv_b[C + g * CL : C + (g + 1) * CL]
        if g == 0:
            ob_eff = ob_eff + o_b
        kwT = kv_w[ch, :].T  # (C, CL): [ko*128+p, sub*128+j] -> [p][sub][ko][j]
        kwT4 = np.ascontiguousarray(
            kwT.reshape(KC, P, 4, P).transpose(1, 2, 0, 3)
        ).astype(BF16NP)
        in_maps.append(
            {
                "x_b": act_layout(x[b]),
                "c_b": act_layout(c[b]),
                "qwT": w_layout(q_w[ch, :].T),
                "kwT": kwT4,
                "vwT": w_layout(kv_w[C + g * CL : C + (g + 1) * CL, :].T),
                "owT": w_layout(o_w[:, ch].T),
                "qb": np.ascontiguousarray(q_b[ch]),
                "kb": np.ascontiguousarray(kv_b[ch]),
                "ob": np.ascontiguousarray(ob_eff.astype(np.float32)),
                "cosr": cosr.astype(BF16NP),
                "sins": sins.astype(BF16NP),
                "pm": pm.astype(BF16NP),
            }
        )
    return in_maps


_NC = None


def _get_nc():
    global _NC
    if _NC is None:
        _NC = _build_program()
    return _NC


def kernel(x, c, q_w, q_b, kv_w, kv_b, o_w, o_b):
    from concourse.bass_utils import run_bass_kernel_spmd

    nc = _get_nc()
    in_maps = make_in_maps(x, c, q_w, q_b, kv_w, kv_b, o_w, o_b)
    res = run_bass_kernel_spmd(nc, in_maps, core_ids=list(range(N_CORES)))
    y = np.empty((B, C, T), np.float32)
    for b in range(B):
        y[b] = res.results[2 * b]["y"] + res.results[2 * b + 1]["y"]
    return y


# revision 29
# speedup vs baseline: 1.1747x; 1.0124x over previous
"""Trainium2 Bass kernel for nn_MultiHeadAttention_52304111731071.

Sharding: 8 cores = 4 batches x 2 head-groups (tensor parallel over heads).
Each core computes q/k/v projections for its 512 channels (8 heads), partial
RoPE, full attention for its heads, and a partial O-projection; the host sums
the two partials per batch.

v7 (161.5us max-core neuron-profile; prior baseline 242.7us; rel err 2.7e-3).
Profile-driven design, measured on these cores:
  - dense back-to-back matmuls pace at ~216ns (N=512 bf16 warm, LDWEIGHTS and
    drain hidden); isolated matmuls cost 379ns - so the kernel is built as a
    single near-gapless PE stream with deep psum/sbuf buffering.
  - ACT exp costs (N+352)/1.2ns per instruction; 8.4M exps/core = 71.5us
    minimum.  Scores for two heads land in one [128, 2x512] psum tile
    spanning two banks and are exp'd by ONE ACT instruction (1114ns).
  - the exp stream starts at ~43us: K proj fully, then per-sub [Q-proj tile,
    scores+exp block] warmup interleaved with the V projection; PV for sub s
    lags scores by 2 subs so the PE never waits on ACT.
  - rope: sin table is pre-permuted on host (sin_perm[p] = sin[pi(p)]) so the
    rotate-half perm-matmul lands the final sin term: epilogue = evict
    (ACT Identity + per-partition bias AP early, DVE tensor_scalar_add during
    attention), sin-mult, perm matmul, cos-mult, add (3 DVE ops).
  - PV psum is evicted to SBUF immediately (frees po banks for the next sub;
    also the reciprocal_approx_fast custom op must read whole offset-0 SBUF
    tiles - offset APs or PSUM reads produce garbage on HW).
  - O-proj(n0) rides as filler inside attention n1 blocks; the O(n1) tail
    pre-accumulates kc0-2 across aux+sc psum slots while the final normalize
    chain runs, so only the kc3 matmuls wait on it; the last normalize skips
    the pvc eviction and reads po/psum directly (only the rcp custom op needs
    SBUF input).  Keeps the PE dense so the HAM clock-gate stays at 2.4GHz.
  - DMA order: wk/c(n0) interleaved, c(n1), then x0/wq BEFORE wv so the Q
    projection is never weight-gated.
PSUM budget (8 banks): scores 2x[128,1024] (4) + misc shp/po (2) + aux (2).

Measured dead ends: fp8e4m3 DoubleRow PV fails the 2e-2 gate (2.06e-2);
split-K PV (K=64 halves, per-member tile_position in one accumulation group)
compiles but dies at runtime with NRT INTERNAL; gpsimd copies in the
normalize chain are 4x slower than DVE and serialize with the broadcasts.
"""

import sys

sys.path.insert(0, "/opt/trn_rl_repo")

import numpy as np
import ml_dtypes

import concourse.bass as bass  # noqa: F401
import concourse.bacc as bacc
import concourse.mybir as mybir
import concourse.tile as tile

B, C, T, H = 4, 1024, 1024, 16
DH = 64
D_ROPE = 32
ROPE_BASE = 10000.0
P = 128
N_CORES = 8
HL = 8  # heads per core
CL = 512  # channels per core
KC = 8  # contraction subtiles (1024/128)
FP32 = mybir.dt.float32
BF16 = mybir.dt.bfloat16
SCALE = 1.0 / 8.0  # 1/sqrt(DH)
BF16NP = ml_dtypes.bfloat16


DEBUG_DUMP = False


def _build_program(repeat=1):
    nc = bacc.Bacc("TRN2", target_bir_lowering=False, debug=False)

    # all big inputs pre-laid-out on host so every DMA line is contiguous
    x_d = nc.dram_tensor("x_b", [P, 2, KC, 512], BF16, kind="ExternalInput")
    c_d = nc.dram_tensor("c_b", [P, 2, KC, 512], BF16, kind="ExternalInput")
    qwT_d = nc.dram_tensor("qwT", [P, KC, CL], BF16, kind="ExternalInput")
    kwT_d = nc.dram_tensor("kwT", [P, 4, KC, P], BF16, kind="ExternalInput")
    vwT_d = nc.dram_tensor("vwT", [P, KC, CL], BF16, kind="ExternalInput")
    owT_d = nc.dram_tensor("owT", [P, 4, C], BF16, kind="ExternalInput")
    qb_d = nc.dram_tensor("qb", [CL], FP32, kind="ExternalInput")
    kb_d = nc.dram_tensor("kb", [CL], FP32, kind="ExternalInput")
    ob_d = nc.dram_tensor("ob", [C], FP32, kind="ExternalInput")
    cos_d = nc.dram_tensor("cosr", [P, T], BF16, kind="ExternalInput")
    sin_d = nc.dram_tensor("sins", [P, T], BF16, kind="ExternalInput")
    pm_d = nc.dram_tensor("pm", [P, P], BF16, kind="ExternalInput")
    y_d = nc.dram_tensor("y", [C, T], FP32, kind="ExternalOutput")

    ID = mybir.ActivationFunctionType.Identity
    EXP = mybir.ActivationFunctionType.Exp
    MUL = mybir.AluOpType.mult
    ADD = mybir.AluOpType.add

    with tile.TileContext(nc) as tc:
      for _rep in range(repeat):
        with (
            tc.tile_pool(name="wq", bufs=1) as wq_p,
            tc.tile_pool(name="wo", bufs=1) as wo_p,
            tc.tile_pool(name="wk", bufs=1) as wk_p,
            tc.tile_pool(name="wv", bufs=1) as wv_p,
            tc.tile_pool(name="acts", bufs=1) as acts,
            tc.tile_pool(name="consts", bufs=1) as consts,
            tc.tile_pool(name="stream", bufs=3) as stream,
            tc.tile_pool(name="rope", bufs=3) as rope_p,
            tc.tile_pool(name="exp", bufs=26) as exp_p,
            tc.tile_pool(name="pvc", bufs=4) as pvc_p,
            tc.tile_pool(name="nrm", bufs=2) as nrm_p,
            tc.tile_pool(name="ysb", bufs=3) as y_p,
        ):
            # ---- big DMAs first: wk + c chunks interleaved so the first
            # K-proj matmuls can start after ~1/4 of the bytes land ----
            wk = wk_p.tile([P, 4, KC, P], BF16, tag="wk")
            ct0 = stream.tile([P, KC, 512], BF16, tag="stream", bufs=3)
            for kc in range(KC):
                nc.sync.dma_start(wk[:, 0, kc, :], kwT_d.ap()[:, 0, kc, :])
                nc.sync.dma_start(ct0[:, kc, :], c_d.ap()[:, 0, kc, :])
            for sub in range(1, 4):
                nc.sync.dma_start(wk[:, sub, :, :], kwT_d.ap()[:, sub, :, :])
            ct1 = stream.tile([P, KC, 512], BF16, tag="stream", bufs=3)
            nc.sync.dma_start(ct1[:], c_d.ap()[:, 1, :, :])
            cts = [ct0, ct1]
            x0 = stream.tile([P, KC, 512], BF16, tag="stream", bufs=3)
            nc.sync.dma_start(x0[:], x_d.ap()[:, 0, :, :])
            wq = wq_p.tile([P, KC, CL], BF16, tag="wq")
            nc.sync.dma_start(wq[:], qwT_d.ap())
            wv = wv_p.tile([P, KC, CL], BF16, tag="wv")
            nc.sync.dma_start(wv[:], vwT_d.ap())
            x1 = stream.tile([P, KC, 512], BF16, tag="stream", bufs=3)
            nc.sync.dma_start(x1[:], x_d.ap()[:, 1, :, :])
            xt = [x0, x1]
            wo = wo_p.tile([P, 4, T], BF16, tag="wo")
            nc.sync.dma_start(wo[:], owT_d.ap())

            # ---- tables / biases / permutation matrix ----
            cosr = consts.tile([P, T], BF16, tag="cosr")
            sins = consts.tile([P, T], BF16, tag="sins")
            nc.gpsimd.dma_start(cosr[:], cos_d.ap())
            nc.gpsimd.dma_start(sins[:], sin_d.ap())
            pm_sb = consts.tile([P, P], BF16, tag="pm")
            nc.gpsimd.dma_start(pm_sb[:], pm_d.ap())
            qb_sb = consts.tile([P, 4], FP32, tag="qb")
            kb_sb = consts.tile([P, 4], FP32, tag="kb")
            ob_sb = consts.tile([P, 8], FP32, tag="ob")
            nc.gpsimd.dma_start(qb_sb[:], qb_d.ap().rearrange("(s p) -> p s", p=P))
            nc.gpsimd.dma_start(kb_sb[:], kb_d.ap().rearrange("(s p) -> p s", p=P))
            nc.gpsimd.dma_start(ob_sb[:], ob_d.ap().rearrange("(s p) -> p s", p=P))

            q_sb = acts.tile([P, 4, T], BF16, tag="qsb")
            k_sb = acts.tile([P, 4, T], BF16, tag="ksb")
            vT_sb = acts.tile([P, KC, HL, 65], BF16, tag="vtsb")
            out_sb = acts.tile([P, 4, T], BF16, tag="osb")
            # ones column per head (col 64 of each 65-col group)
            ones_c = consts.tile([P, KC, 1], BF16, tag="ones")
            nc.any.memset(ones_c[:], 1.0)
            onesr = consts.tile([1, 64], FP32, tag="onesr")
            nc.any.memset(onesr[:], 1.0)
            for j in range(HL):
                nc.vector.tensor_copy(vT_sb[:, :, j, 64:65], ones_c[:])

            with (
                tc.tile_pool(name="scp", bufs=2, space="PSUM") as sc_p,
                tc.tile_pool(name="auxp", bufs=2, space="PSUM") as aux_p,
                tc.tile_pool(name="miscp", bufs=2, space="PSUM") as misc_p,
            ):

                def rope_epilogue(dst, ps, bias_col, n, on_act):
                    """dst (128,512) bf16 slice of q/k subtile: bias + RoPE.

                    tmp = bf16(ps + bias); tmp_s = tmp*sin_perm (sin is
                    pre-permuted on host so the perm matmul lands the final
                    sin term); shp = pm @ tmp_s; dst = tmp*cos + shp.
                    """
                    ncol = slice(n * 512, (n + 1) * 512)
                    tmp = rope_p.tile([P, 512], BF16, tag="tmp", bufs=3)
                    if on_act:
                        nc.scalar.activation(tmp[:], ps[:], ID, bias=bias_col)
                    else:
                        nc.vector.tensor_scalar_add(tmp[:], ps[:], bias_col)
                    ts_ = rope_p.tile([P, 512], BF16, tag="ts", bufs=3)
                    nc.vector.tensor_tensor(ts_[:], tmp[:], sins[:, ncol], MUL)
                    shp = misc_p.tile([P, 512], FP32, tag="misc", name="shp")
                    nc.tensor.matmul(shp[:], pm_sb[:], ts_[:], start=True, stop=True)
                    nc.vector.tensor_tensor(dst, tmp[:], cosr[:, ncol], MUL)
                    nc.vector.tensor_tensor(dst, dst, shp[:], ADD)

                def k_tile(n, sub):
                    ps = aux_p.tile([P, 512], FP32, tag="aux", name="psk")
                    for kc in range(KC):
                        nc.tensor.matmul(
                            ps[:],
                            wk[:, sub, kc, :],
                            cts[n][:, kc, :],
                            start=(kc == 0),
                            stop=(kc == KC - 1),
                        )
                    rope_epilogue(
                        k_sb[:, sub, n * 512 : (n + 1) * 512],
                        ps,
                        kb_sb[:, sub : sub + 1],
                        n,
                        on_act=True,
                    )

                def q_tile(n, sub, on_act):
                    ps = aux_p.tile([P, 512], FP32, tag="aux", name="psq")
                    for kc in range(KC):
                        nc.tensor.matmul(
                            ps[:],
                            wq[:, kc, sub * P : (sub + 1) * P],
                            xt[n][:, kc, :],
                            start=(kc == 0),
                            stop=(kc == KC - 1),
                        )
                    rope_epilogue(
                        q_sb[:, sub, n * 512 : (n + 1) * 512],
                        ps,
                        qb_sb[:, sub : sub + 1],
                        n,
                        on_act=on_act,
                    )

                def v_quarter(mt):
                    ctile = cts[mt // 4]
                    toff = (mt % 4) * P
                    ps = aux_p.tile([P, HL, 64], FP32, tag="aux", name="psv")
                    for kc in range(KC):
                        nc.tensor.matmul(
                            ps[:],
                            ctile[:, kc, toff : toff + P],
                            wv[:, kc, :],
                            start=(kc == 0),
                            stop=(kc == KC - 1),
                        )
                    # one strided copy into the 65-stride vT layout
                    nc.vector.tensor_copy(vT_sb[:, mt, :, 0:64], ps[:])

                if DEBUG_DUMP:
                    _pvdbg = nc.dram_tensor(
                        "pvdbg", [2, 4, 2, 65, 512], FP32, kind="ExternalOutput"
                    )
                    _edbg = nc.dram_tensor(
                        "edbg", [KC, P, 2, 512], BF16, kind="ExternalOutput"
                    )

                # e tiles per (n, sub): list of 8 [P, 2, 512] bf16
                e_store = {}

                def scores_exp(n, sub, tk):
                    ncol = slice(n * 512, (n + 1) * 512)
                    sc = sc_p.tile([P, 2, 512], FP32, tag="sc", name="sc")
                    for half in range(2):
                        hb = half * 64
                        nc.tensor.matmul(
                            sc[:, half, :],
                            k_sb[hb : hb + 64, sub, tk * P : (tk + 1) * P],
                            q_sb[hb : hb + 64, sub, ncol],
                            start=True,
                            stop=True,
                            tile_position=(hb, 0),
                        )
                    e = exp_p.tile([P, 2, 512], BF16, name="e", tag="e", bufs=26)
                    nc.scalar.activation(e[:], sc[:], EXP, scale=SCALE)
                    e_store[(n, sub, tk)] = e
                    if DEBUG_DUMP and (n, sub) == (0, 0):
                        nc.sync.dma_start(_edbg.ap()[tk], e[:])

                po_store = {}

                def pv(n, sub, tk):
                    if tk == 0:
                        po_store[(n, sub)] = [
                            misc_p.tile([65, 512], FP32, tag="misc", name=f"po{h}")
                            for h in range(2)
                        ]
                    po = po_store[(n, sub)]
                    e = e_store[(n, sub, tk)]
                    # NOTE: split-K PV (K=64 halves on row groups 0/64 with
                    # per-group tile_position) crashes the NEFF at runtime
                    # (NRT INTERNAL) - keep full-K PV
                    for half in range(2):
                        j = 2 * sub + half
                        nc.tensor.matmul(
                            po[half][0:65, :],
                            vT_sb[:, tk, j, :],
                            e[:, half, :],
                            start=(tk == 0),
                            stop=(tk == KC - 1),
                        )

                def normalize(n, sub, last=False):
                    ncol = slice(n * 512, (n + 1) * 512)
                    po = po_store.pop((n, sub))
                    if last:
                        # final sub: skip the pvc eviction (nothing needs the
                        # po banks afterwards); den rows via regular DVE copies
                        # (safe from PSUM - only the rcp custom op is not)
                        den = nrm_p.tile([1, 1024], FP32, tag="den", bufs=2)
                        nc.vector.tensor_copy(den[:, 0:512], po[0][64:65, :])
                        nc.vector.tensor_copy(den[:, 512:1024], po[1][64:65, :])
                        rc = nrm_p.tile([1, 1024], FP32, tag="rc", bufs=2)
                        nc.vector.reciprocal_approx_fast(rc[:], den[:])
                        pbt = nrm_p.tile([64, 1024], FP32, tag="pbt", bufs=2)
                        nc.gpsimd.partition_broadcast(pbt[:], rc[:])
                        nc.vector.tensor_tensor(
                            out_sb[0:64, sub, ncol], po[0][0:64, :],
                            pbt[:, 0:512], MUL,
                        )
                        tmp1 = nrm_p.tile([64, 512], BF16, tag="t1", bufs=2)
                        nc.vector.tensor_tensor(
                            tmp1[:], po[1][0:64, :], pbt[:, 512:1024], MUL
                        )
                        nc.sync.dma_start(out_sb[64:128, sub, ncol], tmp1[:])
                        return
                    # evict PV psum to SBUF right away: frees the po banks for
                    # the next sub and keeps the rcp custom op off PSUM
                    pvc = []
                    for h in range(2):
                        t_ = pvc_p.tile([65, 512], FP32, name=f"pvc{h}", tag="pvc", bufs=4)
                        nc.vector.tensor_copy(t_[:], po[h][:])
                        pvc.append(t_)
                        if DEBUG_DUMP:
                            nc.sync.dma_start(_pvdbg.ap()[n, sub, h, :, :], t_[:])
                    # assemble both denominators into a fresh [1, 1024] tile:
                    # reciprocal_approx_fast is a custom DVE op - feed it only
                    # whole tiles at offset 0 (offset APs misbehave on HW)
                    den = nrm_p.tile([1, 1024], FP32, tag="den", bufs=2)
                    nc.vector.tensor_copy(den[:, 0:512], pvc[0][64:65, :])
                    nc.vector.tensor_copy(den[:, 512:1024], pvc[1][64:65, :])
                    rc = nrm_p.tile([1, 1024], FP32, tag="rc", bufs=2)
                    nc.vector.reciprocal_approx_fast(rc[:], den[:])
                    pbt = nrm_p.tile([64, 1024], FP32, tag="pbt", bufs=2)
                    nc.gpsimd.partition_broadcast(pbt[:], rc[:])
                    nc.vector.tensor_tensor(
                        out_sb[0:64, sub, ncol], pvc[0][0:64, :], pbt[:, 0:512], MUL
                    )
                    tmp1 = nrm_p.tile([64, 512], BF16, tag="t1", bufs=2)
                    nc.vector.tensor_tensor(tmp1[:], pvc[1][0:64, :], pbt[:, 512:1024], MUL)
                    # partition-shift via DMA on the (idle) sync queue
                    nc.sync.dma_start(out_sb[64:128, sub, ncol], tmp1[:])

                o_ps = {}

                def o_head(n, m, pool, tag, nkc):
                    # first nkc chunks of the contraction into a fresh psum tile
                    ncol = slice(n * 512, (n + 1) * 512)
                    ps = pool.tile([P, 512], FP32, tag=tag, name="pso")
                    o_ps[(n, m)] = ps
                    for kc in range(nkc):
                        nc.tensor.matmul(
                            ps[:],
                            wo[:, kc, m * P : (m + 1) * P],
                            out_sb[:, kc, ncol],
                            start=(kc == 0),
                            stop=False,
                        )

                def o_finish(n, m):
                    ncol = slice(n * 512, (n + 1) * 512)
                    ps = o_ps.pop((n, m))
                    nc.tensor.matmul(
                        ps[:],
                        wo[:, 3, m * P : (m + 1) * P],
                        out_sb[:, 3, ncol],
                        start=False,
                        stop=True,
                    )
                    ys = y_p.tile([P, 512], FP32, name="ys", tag="ys", bufs=3)
                    nc.vector.tensor_scalar_add(ys[:], ps[:], ob_sb[:, m : m + 1])
                    nc.sync.dma_start(y_d.ap()[m * P : (m + 1) * P, ncol], ys[:])

                def o_tile(n, m):
                    o_head(n, m, aux_p, "aux", 3)
                    o_finish(n, m)

                def block(sc_ns, pv_ns):
                    """Lag-1 attention block: scores+exp for sc_ns while PV for
                    pv_ns accumulates; interleaved per tk."""
                    for tk in range(KC):
                        if sc_ns is not None:
                            scores_exp(sc_ns[0], sc_ns[1], tk)
                        if pv_ns is not None:
                            pv(pv_ns[0], pv_ns[1], tk)

                # ================= emission =================
                # projections: K fully, then Q(n0) tiles feeding the warm-up
                for n in range(2):
                    for sub in range(4):
                        k_tile(n, sub)
                q_tile(0, 0, on_act=True)
                block((0, 0), None)
                q_tile(0, 1, on_act=True)
                for mt in range(4):
                    v_quarter(mt)
                block((0, 1), None)
                # V q4-7 interleaved with PV(0,0): merges the two serial
                # phases that caused the exp-stream transition stall
                for mt in range(4, 8):
                    v_quarter(mt)
                    pv(0, 0, 2 * (mt - 4))
                    pv(0, 0, 2 * (mt - 4) + 1)
                normalize(0, 0)
                q_tile(0, 2, on_act=False)
                block((0, 2), None)
                q_tile(0, 3, on_act=False)

                # steady pipeline: PV lags scores by 2 subs
                blocks = [
                    ((0, 3), (0, 1), [lambda: q_tile(1, 0, on_act=False)]),
                    ((1, 0), (0, 2), [lambda: q_tile(1, 1, on_act=False)]),
                    ((1, 1), (0, 3), [lambda: q_tile(1, 2, on_act=False)]),
                    ((1, 2), (1, 0), [lambda: q_tile(1, 3, on_act=False),
                                      lambda: o_tile(0, 0)]),
                    ((1, 3), (1, 1), [lambda: o_tile(0, 1), lambda: o_tile(0, 2)]),
                    (None, (1, 2), [lambda: o_tile(0, 3), lambda: o_tile(0, 4),
                                    lambda: o_tile(0, 5), lambda: o_tile(0, 6),
                                    lambda: o_tile(0, 7)]),
                ]
                for sc_ns, pv_ns, fills in blocks:
                    block(sc_ns, pv_ns)
                    normalize(*pv_ns)
                    for f in fills:
                        f()

                # trailing PV(1,3) interleaved with the remaining O(n0) tiles
                # and O(n1) kc0-2 pre-accumulation, so the PE stays dense (and
                # the HAM clock warm) through the tail; only the kc3 matmuls
                # wait on the final normalize
                for tk in range(KC):
                    pv(1, 3, tk)
                # kc0-2 pre-accumulation overlaps the final normalize chain
                o_head(1, 0, aux_p, "aux", 3)
                o_head(1, 1, aux_p, "aux", 3)
                o_head(1, 2, sc_p, "sc", 3)
                o_head(1, 3, sc_p, "sc", 3)
                normalize(1, 3, last=True)
                o_head(1, 4, misc_p, "misc", 3)
                o_head(1, 5, misc_p, "misc", 3)
                o_finish(1, 0)
                o_finish(1, 1)
                o_finish(1, 2)
                o_finish(1, 3)
                o_finish(1, 4)
                o_finish(1, 5)
                o_tile(1, 6)
                o_tile(1, 7)

    nc.compile()
    return nc


def _rope_tables():
    theta = 1.0 / (ROPE_BASE ** (np.arange(0, D_ROPE, 2, dtype=np.float32) / D_ROPE))
    ang = np.arange(T, dtype=np.float32)[:, None] * theta[None, :]  # (T, 16)
    ang2 = np.concatenate([ang, ang], axis=1)  # (T, 32)
    cos2 = np.cos(ang2).astype(np.float32)  # (T, 32)
    sin2 = np.sin(ang2).astype(np.float32)
    cosr = np.ones((P, T), np.float32)
    sins = np.zeros((P, T), np.float32)
    for base in (0, 64):
        for d in range(D_ROPE):
            cosr[base + d] = cos2[:, d]
            # permuted sin: row p holds sin[pi(p)] where pi swaps d <-> d+16
            # within each 32-row rope block, so that
            # (pm @ (tmp * sin_perm))[r] = sign_r * tmp[pi(r)] * sin[r]
            dp = d + 16 if d < 16 else d - 16
            sins[base + d] = sin2[:, dp]
    # permutation matrix: sh = pm.T @ tmp; sh[g+i] = -tmp[g+16+i],
    # sh[g+16+i] = +tmp[g+i] for i in 0:16, g in {0,64}; zero elsewhere
    pm = np.zeros((P, P), np.float32)
    for g in (0, 64):
        for i in range(16):
            pm[g + 16 + i, g + i] = -1.0
            pm[g + i, g + 16 + i] = 1.0
    return cosr, sins, pm


def make_in_maps(x, c, q_w, q_b, kv_w, kv_b, o_w, o_b):
    x = np.asarray(x, np.float32)
    c = np.asarray(c, np.float32)
    q_w = np.asarray(q_w, np.float32)
    q_b = np.asarray(q_b, np.float32)
    kv_w = np.asarray(kv_w, np.float32)
    kv_b = np.asarray(kv_b, np.float32)
    o_w = np.asarray(o_w, np.float32)
    o_b = np.asarray(o_b, np.float32)
    cosr, sins, pm = _rope_tables()

    def act_layout(a):  # (C, T) -> (P, 2, KC, 512): [p][n][ko][t]
        return np.ascontiguousarray(
            a.reshape(KC, P, 2, 512).transpose(1, 2, 0, 3)
        ).astype(BF16NP)

    def w_layout(wT):  # (C, CL) -> (P, KC, CL): [p][ko][m]
        ko = wT.shape[0] // P
        return np.ascontiguousarray(
            wT.reshape(ko, P, wT.shape[1]).transpose(1, 0, 2)
        ).astype(BF16NP)

    in_maps = []
    for core in range(N_CORES):
        b, g = core // 2, core % 2
        ch = slice(g * CL, (g + 1) * CL)
        ob_eff = o_w[:, ch] @ kv_b[C + g * CL : C + (g + 1) * CL]
        if g == 0:
            ob_eff = ob_eff + o_b
        kwT = kv_w[ch, :].T  # (C, CL): [ko*128+p, sub*128+j] -> [p][sub][ko][j]
        kwT4 = np.ascontiguousarray(
            kwT.reshape(KC, P, 4, P).transpose(1, 2, 0, 3)
        ).astype(BF16NP)
        in_maps.append(
            {
                "x_b": act_layout(x[b]),
                "c_b": act_layout(c[b]),
                "qwT": w_layout(q_w[ch, :].T),
                "kwT": kwT4,
                "vwT": w_layout(kv_w[C + g * CL : C + (g + 1) * CL, :].T),
                "owT": w_layout(o_w[:, ch].T),
                "qb": np.ascontiguousarray(q_b[ch]),
                "kb": np.ascontiguousarray(kv_b[ch]),
                "ob": np.ascontiguousarray(ob_eff.astype(np.float32)),
                "cosr": cosr.astype(BF16NP),
                "sins": sins.astype(BF16NP),
                "pm": pm.astype(BF16NP),
            }
        )
    return in_maps


_NC = None


def _get_nc():
    global _NC
    if _NC is None:
        _NC = _build_program()
    return _NC


def kernel(x, c, q_w, q_b, kv_w, kv_b, o_w, o_b):
    from concourse.bass_utils import run_bass_kernel_spmd

    nc = _get_nc()
    in_maps = make_in_maps(x, c, q_w, q_b, kv_w, kv_b, o_w, o_b)
    res = run_bass_kernel_spmd(nc, in_maps, core_ids=list(range(N_CORES)))
    y = np.empty((B, C, T), np.float32)
    for b in range(B):
        y[b] = res.results[2 * b]["y"] + res.results[2 * b + 1]["y"]
    return y
